# revision 49
# baseline (speedup 1.0000x reference)
"""AAConv (attention-augmented conv) Trainium2 kernel, 8-core data-parallel.

Reference shapes: x (16,256,32,32) f32
  conv branch: 3x3 SAME conv 256->128 (+bias)
  attn branch: 1x1 qkv conv (k|q|v = 128|128|128 rows of qkv_w), 8 heads d=16,
               softmax attention over 1024 positions, 1x1 proj 128->128 (+bias)
  out = concat([conv_out, attn_out], axis=1) -> (16,256,32,32)

Sharding: pure data-parallel over batch. Each of 8 cores gets 2 images and
all weights; outputs concatenated on host.

Per-core design (channels on partitions, pixels on free dim):
 - kq/vt 1x1-conv matmuls in fp8e4 DoubleRow (both ct contraction blocks in
   one half-rate pass; dst partition 0 so the ISA check passes). K/Q stored
   padded: head h at partitions 32h..32h+16, produced by M=128 matmuls
   against zero-padded transposed weights.
 - logits transposed, L^T[k,q] (lhsT=K_h [16,128], rhs=Q_h [16,512]),
   row-tiled over heads (tile_position=(32hp,0)); lg tiles [128, 2x512]
   hold a kt-pair of one head.
 - exp out of PSUM into fp8e5 st tiles: ~3/4 on ScalarE (AF.Exp), 1/4 on
   DVE via the int8 bitcast-linear trick (bits = lg*4*SCALE/ln2 + 60.5;
   e5m2 covers the full logit range with >15 sigma of margin). The early
   (tp==0) tiles go to DVE so its round tail stays clear.
 - AV in fp8e5 DoubleRow, per-head rounds: av32 [32,512] accumulates one
   (head, q-half) at PSUM partition 0 (rows 0:16 attn, 16 den via the vt
   ones/pad columns = 1.0). Four heads' av32 tiles are DMA-assembled
   (SBUF->SBUF moves partitions) into a 128-row avq, then one
   reciprocal_approx_fast + stream_shuffle([16]*32) + multiply normalizes
   a whole quadrant group at full DVE width.
 - conv rhs as single contiguous runs over a zero-padded 34-wide flat
   space (two const-initialized alternating pad buffers), in 15/15/2-row
   chunks interleaved between attention rounds so the PE never idles
   against the exp backlog; junk columns skipped on evacuation.
 - proj with zero-padded transposed weights over the assembled attn layout
   (pad rows exactly 0 so junk rows never reach the output).
 - software-pipelined schedule: in(i+2) / pre(i+1) staged inside image i's
   rounds; biases ride the Pool SWDGE queue; weight DMAs batched (conv_w as
   2 big transposing DMAs) and spread across SP/ACT/Pool queues; prologue
   PSUM tiles rotate through 3 slot groups. PSUM: lg 3x2 banks + av 1 + ms 1.
 - TimelineSim model: ~137us single-shot (baseline was ~203us); engine busy
   ~102us DVE / ~101us ACT / ~90us PE. HW rel err 2.4e-3 (conv-dominated).
"""

import sys

for p in ("/opt/trn_rl_repo",):
    if p not in sys.path:
        sys.path.insert(0, p)

import numpy as np

import concourse.tile as tile
from concourse import bacc, mybir
from concourse.masks import make_identity

F32 = mybir.dt.float32
BF16 = mybir.dt.bfloat16
I16 = mybir.dt.int16
AF = mybir.ActivationFunctionType
ALU = mybir.AluOpType

# Problem dims (hardcoded)
B, C, H, W = 16, 256, 32, 32
HW = H * W                      # 1024
CO, DK, DV, NH = 256, 128, 128, 8
D = DK // NH                    # 16 head dim
CONV_CO = CO - DV               # 128
N_CORES = 8
BL = B // N_CORES               # 2 images per core
HP = H + 2                      # 34 padded
PADHW = HP * HP                 # 1156
SCALE = float(D) ** -0.5        # 0.25
import os as _os_
LG_BUFS = int(_os_.environ.get("AACONV_LG_BUFS", "3"))
MS_BUFS = int(_os_.environ.get("AACONV_MS_BUFS", "1"))
AV_BUFS = int(_os_.environ.get("AACONV_AV_BUFS", "1"))
FP8 = mybir.dt.float8e5
I8 = mybir.dt.int8
DR = mybir.MatmulPerfMode.DoubleRow
# int8-bitcast exp for e5m2: bits = lg*4*SCALE/ln2 + (15*4 + 0.5)
EXA5 = 4.0 / 0.6931471805599453 * SCALE
EXB5 = 60.5
ST_DT = FP8
# fp8e4 DoubleRow for the kq/vt matmuls (dst partition 0 -> legal): pairs
# the two ct contraction blocks into one half-rate pass
FP8_KQ = True
FP8E4 = mybir.dt.float8e4
KQ_DT = FP8E4


def build_nc():
    nc = bacc.Bacc("TRN2", target_bir_lowering=False, debug=False,
                   num_devices=N_CORES)

    x_ext = nc.declare_dram_parameter("x", [BL, C, HW], F32, isOutput=False)
    convw_ext = nc.declare_dram_parameter("conv_w", [9, C, CONV_CO], F32, isOutput=False)
    convb_ext = nc.declare_dram_parameter("conv_b", [1, CONV_CO], F32, isOutput=False)
    qkvw_ext = nc.declare_dram_parameter("qkv_w", [2 * DK + DV, C], F32, isOutput=False)
    qkvb_ext = nc.declare_dram_parameter("qkv_b", [1, 2 * DK + DV], F32, isOutput=False)
    projw_ext = nc.declare_dram_parameter("proj_w", [DV, DV], F32, isOutput=False)
    projb_ext = nc.declare_dram_parameter("proj_b", [1, DV], F32, isOutput=False)
    out_ext = nc.declare_dram_parameter("out", [BL, CO, HW], F32, isOutput=True)

    with tile.TileContext(nc) as tc:
        with (
            tc.tile_pool(name="const", bufs=1) as constp,
            tc.tile_pool(name="stage", bufs=1) as stagep,
            tc.tile_pool(name="img", bufs=3) as imgp,
            tc.tile_pool(name="st", bufs=8) as stp,
            tc.tile_pool(name="psum", bufs=1, space="PSUM") as psp,
        ):
            # ---------------- weights + input staging ----------------
            # Latency-ordered: x DMAs first on SP/HWDGE, then qkv weights
            # (they gate the whole PE chain), biases on the Pool SWDGE,
            # conv/proj weights last (needed later). Prologue PSUM tiles
            # alternate between the "ms" and "av" slots so matmul->evac
            # chains double-buffer.
            ident = constp.tile([128, 128], F32)
            make_identity(nc, ident[:])
            # touch Exp early so the ~2.7us ACT table-set load overlaps the
            # input DMAs instead of the first attention round
            actwarm = stagep.tile([1, 8], F32)
            nc.scalar.activation(actwarm[:], ident[0:1, 0:8], AF.Exp)

            import os as _os
            _reps = int(_os.environ.get("AACONV_BENCH_REPS", "1"))
            n_imgs = BL * _reps
            imgs = {}

            def stage_in(img):
                xin = imgp.tile([128, 2 * HW], F32, tag="xin", name=f"xin_{img}")
                for ct in range(2):
                    nc.sync.dma_start(
                        xin[:, ct * HW:(ct + 1) * HW],
                        x_ext[img % BL, ct * 128:(ct + 1) * 128, :])
                x_bf = imgp.tile([128, 2 * HW], KQ_DT, tag="xbf",
                                 name=f"xbf_{img}")
                if img == 0:
                    # critical path: compact on DVE, per ct half so the first
                    # half converts while the second is still in flight
                    for ct in range(2):
                        nc.vector.tensor_copy(x_bf[:, ct * HW:(ct + 1) * HW],
                                              xin[:, ct * HW:(ct + 1) * HW])
                else:
                    nc.gpsimd.tensor_copy(x_bf[:], xin[:])
                imgs[img] = {"x_bf": x_bf, "xin": xin}

            stage_in(0)
            # qkv weights next on HWDGE (they gate the PE transpose chain)
            qkvw_sb = stagep.tile([128, 3 * C], F32)  # blk b at cols b*256
            for blk in range(3):
                nc.scalar.dma_start(
                    qkvw_sb[:, blk * C:(blk + 1) * C],
                    qkvw_ext[blk * 128:(blk + 1) * 128, :],
                )
            if n_imgs > 1:
                stage_in(1)
            # bias DMAs early on the Pool SWDGE queue
            qkvb_sb = stagep.tile([1, 2 * DK + DV], F32)
            nc.gpsimd.dma_start(qkvb_sb[:], qkvb_ext[:])
            bv_f32 = stagep.tile([1, DV], F32)
            nc.gpsimd.dma_start(bv_f32[:], qkvb_ext[:, 2 * DK:])
            projb_f32 = stagep.tile([1, DV], F32)
            nc.gpsimd.dma_start(projb_f32[:], projb_ext[:])

            def _pp(i, sz=(128, 128)):
                # rotating prologue PSUM slot (ms / av / one lg slot)
                tag, bufs = (("ms", MS_BUFS), ("av", AV_BUFS),
                             ("lg", LG_BUFS))[i % 3]
                return psp.tile(list(sz), F32, tag=tag, bufs=bufs,
                                name=f"pp_{_pp.n}")
            _pp.n = 0

            # wkq_pad: [128 c, ct*512 + tgt*256 + hh*128 + hp*32 + d] bf16, zero pad
            wkq_pad = constp.tile([128, 2 * 512], KQ_DT)
            nc.vector.memset(wkq_pad[:], 0.0)
            wvT = constp.tile([128, 2 * 128], KQ_DT)
            for i, (ct, blk) in enumerate((c, b) for c in range(2)
                                          for b in range(3)):
                tps = _pp(i)
                _pp.n += 1
                nc.tensor.transpose(
                    tps[:], qkvw_sb[:, blk * C + ct * 128: blk * C + (ct + 1) * 128],
                    ident[:])
                if blk < 2:
                    dst = wkq_pad[:, ct * 512 + blk * 256:
                                  ct * 512 + (blk + 1) * 256].rearrange(
                        "p (h d) -> p h d", d=32)[:, :, 0:16]
                    src = tps[:].rearrange("p (h d) -> p h d", d=16)
                    nc.vector.tensor_copy(dst, src)
                else:
                    nc.vector.tensor_copy(
                        wvT[:, ct * 128:(ct + 1) * 128], tps[:])

            # --- bias columns, built via PE (no SBUF-writing DMAs) ---
            # bias ROWS first (free-dim scatters, DVE-legal), then K=1
            # matmuls against ones[1,1] turn each row into a column.
            ones11 = constp.tile([1, 1], BF16)
            nc.vector.memset(ones11[:], 1.0)
            brow_pad = constp.tile([1, 512], BF16)
            nc.vector.memset(brow_pad[:], 0.0)
            for tgt in range(2):
                for hh in range(2):
                    nc.vector.tensor_copy(
                        brow_pad[0:1, (tgt * 2 + hh) * 128:
                                 (tgt * 2 + hh + 1) * 128].rearrange(
                            "p (a b) -> p a b", b=32)[:, :, 0:16],
                        qkvb_sb[0:1, tgt * DK + 64 * hh: tgt * DK + 64 * (hh + 1)
                                ].rearrange("p (a b) -> p a b", b=16))
            pre_ps = psp.tile([128, 512], F32, tag="av", bufs=AV_BUFS,
                              name="pre_ps")
            for blk in range(4):
                nc.tensor.matmul(pre_ps[:, blk:blk + 1],
                                 brow_pad[0:1, blk * 128:(blk + 1) * 128],
                                 ones11[0:1, :], start=True, stop=True,
                                 skip_group_check=True)
            bias_cols = constp.tile([128, 6], F32)
            nc.vector.tensor_copy(bias_cols[:, 0:4], pre_ps[:, 0:4])

            # v-bias broadcast to 128 partitions via PE (ones ⊗ bv)
            bv_bf = stagep.tile([1, DV], BF16)
            nc.vector.tensor_copy(bv_bf[:], bv_f32[:])
            ones_row = constp.tile([1, 128], BF16)
            nc.vector.memset(ones_row[:], 1.0)
            nc.tensor.matmul(pre_ps[:, 384:512], ones_row[:], bv_bf[:],
                             start=True, stop=True, skip_group_check=True)
            bv_bc = constp.tile([128, 128], F32)
            nc.vector.tensor_copy(bv_bc[:], pre_ps[:, 384:512])

            # conv bias + conv weights (ACT queue; big DMAs last)
            convb_f32 = stagep.tile([1, CONV_CO], F32)
            nc.scalar.dma_start(convb_f32[:], convb_ext[:])
            convb_row = constp.tile([1, CONV_CO], BF16)
            nc.vector.tensor_copy(convb_row[:], convb_f32[:])
            nc.tensor.matmul(pre_ps[:, 4:5], convb_row[0:1, :], ones11[0:1, :],
                             start=True, stop=True, skip_group_check=True)
            nc.vector.tensor_copy(bias_cols[:, 4:5], pre_ps[:, 4:5])
            projb_row = constp.tile([1, DV], BF16)
            nc.vector.tensor_copy(projb_row[:], projb_f32[:])
            nc.tensor.matmul(pre_ps[:, 5:6], projb_row[0:1, :], ones11[0:1, :],
                             start=True, stop=True, skip_group_check=True)
            nc.vector.tensor_copy(bias_cols[:, 5:6], pre_ps[:, 5:6])

            # conv weights: natural [c, o] per tap, bf16. cols (ct*9+t)*128+o.
            # One big DMA per ct half (src AP transposes t<->c).
            wconv_f32 = stagep.tile([128, 2 * 9 * CONV_CO], F32)
            wconv = constp.tile([128, 2 * 9 * CONV_CO], BF16)
            for ct in range(2):
                blk = slice(ct * 9 * CONV_CO, (ct + 1) * 9 * CONV_CO)
                nc.scalar.dma_start(
                    wconv_f32[:, blk].rearrange("p (t o) -> p t o", t=9),
                    convw_ext[:, ct * 128:(ct + 1) * 128, :].rearrange(
                        "t c o -> c t o"))
                nc.vector.tensor_copy(wconv[:, blk], wconv_f32[:, blk])

            # proj weights (needed only at the first proj, ~late)
            projw_sb = stagep.tile([128, 128], F32)
            nc.scalar.dma_start(projw_sb[:], projw_ext[:])
            # padded projT: rows 32hp+d = proj_w^T row (4hh+hp)*16+d, rest 0
            # (matches attn_pad where head hp's attn lives at rows 32hp..+16)
            projw_pad = stagep.tile([128, 2 * 128], F32)
            nc.vector.memset(projw_pad[:], 0.0)
            for hh in range(2):
                nc.vector.tensor_copy(
                    projw_pad[:, hh * 128:(hh + 1) * 128].rearrange(
                        "p (a b) -> p a b", b=32)[:, :, 0:16],
                    projw_sb[:, 64 * hh:64 * (hh + 1)].rearrange(
                        "p (a b) -> p a b", b=16))
            projT_pad = constp.tile([128, 2 * 128], BF16)
            for hh in range(2):
                tps2 = _pp(hh)
                _pp.n += 1
                nc.tensor.transpose(
                    tps2[:], projw_pad[:, hh * 128:(hh + 1) * 128], ident[:])
                nc.vector.tensor_copy(projT_pad[:, hh * 128:(hh + 1) * 128],
                                      tps2[:])

            # vt_aug / xpad double buffers: the ones/pad columns of vt and
            # the zero border of xpad never change, so initialize two
            # alternating buffers once instead of re-memsetting per image.
            # Buffer 0 (image 0, needed soon) on Pool right after the bias
            # DMAs; buffer 1 later.
            PADW_ = PADHW + 36
            vt_bufs, xpad_bufs = [], []
            for b in range(2):
                vtb = constp.tile([128, 2 * 8 * 128], ST_DT, name=f"vtb_{b}")
                nc.gpsimd.memset(vtb[:], 1.0)
                vt_bufs.append(vtb)
                xpb = constp.tile([128, 2 * PADW_], BF16, name=f"xpb_{b}")
                nc.gpsimd.memset(xpb[:], 0.0)
                xpad_bufs.append(xpb)

            # ---------------- per image, software-pipelined ----------------
            # Stages are emitted in an interleaved order so the PE always has
            # conv / next-image kq work queued while ACT+DVE chew on the
            # current round's exp backlog:
            #   pre(0), [round r; conv chunk r]*, in(i+2), round 3, pre(i+1),
            #   proj(i), ...
            PADW = PADHW + 36   # room for the last conv chunk's shifted reads
            CHUNKS = ((0, 15), (15, 15), (30, 2))
            pend = [None]    # deferred normalize tail (global across images)

            def normalize(avq, avc, rrec, attn_pad, sl, slh, tag):
                # avq rows per quadrant hp (assembled from 4 per-head av32
                # tiles by SBUF-to-SBUF DMAs): 32hp..+16 = unnormalized attn,
                # 32hp+16 = den, +17..32 = den copies (vt pad cols = 1.0);
                # all rows are far from zero so the fast reciprocal stays
                # finite. Broadcast each quadrant's den-row reciprocal with
                # one stream_shuffle, then one multiply normalizes while
                # downcasting to bf16.
                nc.vector.reciprocal_approx_fast(rrec[:, sl], avq[:])
                nc.vector.stream_shuffle(avc[:, sl], rrec[:, sl], [16] * 32)
                nc.vector.tensor_mul(attn_pad[:, slh], avq[:], avc[:, sl])

            def stage_pre(img):
                s = imgs[img]
                x_bf = s["x_bf"]

                # ---- K_pad / Q_pad ----
                k_pad = imgp.tile([128, 2 * HW], BF16, tag="kpad", name=f"kpad_{img}")
                q_pad = imgp.tile([128, 2 * HW], BF16, tag="qpad", name=f"qpad_{img}")
                # vt_aug block (hh,kt) at cols (hh*8+kt)*128 + hp*32 +
                #   [0 = ones, 1:16 = ones, 16:32 = V_h]  (M=32 AV matmuls
                #   write full PSUM quadrants; denominator lands on quadrant
                #   rows 32hp, attn on rows 32hp+16..32). Interleaved with the
                #   kq units so round 0's first AV sees vt kt0-3 early.
                vt_aug = vt_bufs[img % 2]

                wkq4 = wkq_pad[:].rearrange("p (ct blk) -> p ct blk", ct=2)
                xbf3 = x_bf[:].rearrange("p (ct n) -> p ct n", ct=2)

                def kq_unit(i, hh, tgt, qn):
                    dst = k_pad if tgt == 0 else q_pad
                    kqps = _pp(i, (128, 512))
                    _pp.n += 1
                    if FP8_KQ:
                        nc.tensor.matmul(
                            kqps[:],
                            wkq4[:, :, tgt * 256 + hh * 128:
                                 tgt * 256 + (hh + 1) * 128],
                            xbf3[:, :, qn * 512:(qn + 1) * 512],
                            start=True, stop=True, perf_mode=DR)
                    else:
                        for ct in range(2):
                            nc.tensor.matmul(
                                kqps[:],
                                wkq_pad[:, ct * 512 + tgt * 256 + hh * 128:
                                        ct * 512 + tgt * 256 + (hh + 1) * 128],
                                x_bf[:, ct * HW + qn * 512:
                                     ct * HW + (qn + 1) * 512],
                                start=(ct == 0), stop=(ct == 1))
                    nc.vector.tensor_scalar(
                        dst[:, hh * HW + qn * 512:
                            hh * HW + (qn + 1) * 512], kqps[:],
                        bias_cols[:, tgt * 2 + hh: tgt * 2 + hh + 1],
                        None, ALU.add)

                wvT3 = wvT[:].rearrange("p (ct d) -> p ct d", ct=2)

                def vt_unit(i, kt):
                    vtps = _pp(i)
                    _pp.n += 1
                    if FP8_KQ:
                        nc.tensor.matmul(
                            vtps[:],
                            xbf3[:, :, kt * 128:(kt + 1) * 128],
                            wvT3[:],
                            start=True, stop=True, perf_mode=DR)
                    else:
                        for ct in range(2):
                            nc.tensor.matmul(
                                vtps[:],
                                x_bf[:, ct * HW + kt * 128:
                                     ct * HW + (kt + 1) * 128],
                                wvT[:, ct * 128:(ct + 1) * 128],
                                start=(ct == 0), stop=(ct == 1))
                    dst = vt_aug[:].rearrange(
                        "p (hh kt h d) -> p hh kt h d",
                        hh=2, kt=8, d=32)[:, :, kt, :, 0:16]
                    srcv = vtps[:].rearrange(
                        "p (hh h d) -> p hh h d", hh=2, d=16)
                    bvb = bv_bc[:].rearrange(
                        "p (hh h d) -> p hh h d", hh=2, d=16)
                    nc.vector.tensor_add(dst, srcv, bvb)

                ui = 0
                for hh in range(2):
                    for tgt in range(2):
                        for qn in range(2):
                            kq_unit(ui, hh, tgt, qn)
                            ui += 1
                    for kt in range(4 * hh, 4 * hh + 4):
                        vt_unit(ui, kt)
                        ui += 1

                # zero-padded 34x34 layout for the conv, filled via DVE
                # (emitted after kq/vt so it doesn't delay the first round)
                xpad = xpad_bufs[img % 2]
                xsrc = s["xin"] if FP8_KQ else x_bf
                for ct in range(2):
                    nc.gpsimd.tensor_copy(
                        xpad[:, ct * PADW: ct * PADW + PADHW].rearrange(
                            "p (h w) -> p h w", h=HP)[:, 1:33, 1:33],
                        xsrc[:, ct * HW:(ct + 1) * HW].rearrange(
                            "p (h w) -> p h w", h=H))
                s["xpad"] = xpad
                s["k_pad"], s["q_pad"], s["vt_aug"] = k_pad, q_pad, vt_aug
                s["out_conv"] = imgp.tile([128, HW], F32, tag="oconv",
                                          name=f"oconv_{img}")
                s["attn_pad"] = imgp.tile([128, 2 * HW], BF16, tag="attnp",
                                          name=f"attnp_{img}")
                s["avc"] = imgp.tile([128, HW], F32, tag="avc", name=f"avc_{img}")
                s["rrec"] = imgp.tile([128, HW], F32, tag="rrec", name=f"rrec_{img}")
                s["rrec_bf"] = None

            def stage_conv(img, ci):
                s = imgs[img]
                r0, nr = CHUNKS[ci]
                n = (nr - 1) * HP + W          # chunk free size (<=512)
                cs = (r0 + 1) * HP + 1         # pad-flat offset of (r0, 0)
                cvps = psp.tile([128, 512], F32, tag="ms", bufs=MS_BUFS,
                                name=f"cvps_{img}_{r0}")
                for t in range(9):
                    dy, dx = t // 3, t % 3
                    sh = (dy - 1) * HP + (dx - 1)
                    for ct in range(2):
                        nc.tensor.matmul(
                            cvps[:, 0:n],
                            wconv[:, (ct * 9 + t) * 128:(ct * 9 + t + 1) * 128],
                            s["xpad"][:, ct * PADW + cs + sh:
                                      ct * PADW + cs + sh + n],
                            start=((t, ct) == (0, 0)), stop=((t, ct) == (8, 1)))
                nc.vector.tensor_scalar(
                    s["out_conv"][:, r0 * W:(r0 + nr) * W].rearrange(
                        "p (h w) -> p h w", h=nr),
                    cvps[:, 0:nr * HP].rearrange(
                        "p (h w) -> p h w", w=HP)[:, :, 0:W],
                    bias_cols[:, 4:5], None, ALU.add)
                nc.sync.dma_start(
                    out_ext[img % BL, 0:CONV_CO, r0 * W:(r0 + nr) * W],
                    s["out_conv"][:, r0 * W:(r0 + nr) * W])

            def stage_round(img, ridx, mid=None):
                # one round = one (head, q-half): av32 accumulates at PSUM
                # partition 0 so the fp8e5 DoubleRow AV (2 kt blocks/pass)
                # passes the dst-partition ISA check.
                s = imgs[img]
                grp, hp = divmod(ridx, 4)
                hh, qh = divmod(grp, 2)
                h = 4 * hh + hp
                k_pad, q_pad, vt_aug = s["k_pad"], s["q_pad"], s["vt_aug"]
                sl = slice(qh * 512, (qh + 1) * 512)
                av = psp.tile([32, 512], F32, tag="av", bufs=AV_BUFS,
                              name=f"av_{img}_{h}_{qh}")
                vt5 = vt_aug[:].rearrange("p (vhh kt vh d) -> p vhh kt vh d",
                                          vhh=2, kt=8, d=32)
                for tp in range(4):
                    lg = psp.tile([128, 1024], F32, tag="lg", bufs=LG_BUFS,
                                  name=f"lg_{img}_{h}_{qh}_{tp}")
                    for i in range(2):
                        nc.tensor.matmul(
                            lg[:, i * 512:(i + 1) * 512],
                            k_pad[32 * hp:32 * hp + 16,
                                  hh * HW + (2 * tp + i) * 128:
                                  hh * HW + (2 * tp + i + 1) * 128],
                            q_pad[32 * hp:32 * hp + 16,
                                  hh * HW + qh * 512: hh * HW + (qh + 1) * 512],
                            start=True, stop=True,
                            tile_position=(32 * hp, 0))
                    st = stp.tile([128, 1024], ST_DT, tag="st",
                                  name=f"st_{img}_{h}_{qh}_{tp}")
                    if (tp == 0 and hp != 0) or (tp == 2 and hp == 3):
                        nc.vector.tensor_scalar(
                            st[:].bitcast(I8), lg[:],
                            EXA5, EXB5, ALU.mult, ALU.add)
                    else:
                        nc.scalar.activation(st[:], lg[:], AF.Exp,
                                             scale=SCALE)
                    nc.tensor.matmul(
                        av[:],
                        vt5[:, hh, 2 * tp:2 * tp + 2, hp, :],
                        st[:].rearrange("p (two q) -> p two q", two=2),
                        start=(tp == 0), stop=(tp == 3),
                        perf_mode=DR, skip_group_check=True)
                    if tp == 0 and pend[0] is not None:
                        normalize(*pend[0])
                        pend[0] = None
                    if tp == 1 and mid is not None:
                        mid()
                # evacuate av to SBUF right away so the PSUM bank frees
                # (alternating ACT/DVE), then DMA the 32 rows into quadrant
                # hp of the group's 128-row avq tile; the deferred normalize
                # fires once all 4 heads of the (hh, qh) group are in.
                av_sb = imgp.tile([32, 512], F32, tag="avsb",
                                  name=f"avsb_{img}_{h}_{qh}")
                nc.vector.tensor_copy(av_sb[:], av[:])
                if hp == 0:
                    s["avq"] = imgp.tile([128, 512], F32, tag="avq",
                                         name=f"avq_{img}_{hh}_{qh}")
                nc.gpsimd.dma_start(s["avq"][32 * hp:32 * hp + 32, :],
                                    av_sb[:])
                if hp == 3:
                    slh = slice(hh * HW + qh * 512, hh * HW + (qh + 1) * 512)
                    pend[0] = (s["avq"], s["avc"], s["rrec"], s["attn_pad"],
                               sl, slh, f"{img}_{hh}_{qh}")

            def stage_proj(img):
                s = imgs[img]
                if pend[0] is not None:
                    normalize(*pend[0])
                    pend[0] = None
                out_proj = imgp.tile([128, HW], F32, tag="oproj",
                                     name=f"oproj_{img}")
                for qn in range(2):
                    projps = psp.tile([128, 512], F32, tag="ms", bufs=MS_BUFS,
                                      name=f"projps_{img}_{qn}")
                    for hh in range(2):
                        nc.tensor.matmul(
                            projps[:],
                            projT_pad[:, hh * 128:(hh + 1) * 128],
                            s["attn_pad"][:, hh * HW + qn * 512:
                                          hh * HW + (qn + 1) * 512],
                            start=(hh == 0), stop=(hh == 1))
                    nc.vector.tensor_scalar(
                        out_proj[:, qn * 512:(qn + 1) * 512], projps[:],
                        bias_cols[:, 5:6], None, ALU.add)
                    nc.sync.dma_start(
                        out_ext[img % BL, CONV_CO:, qn * 512:(qn + 1) * 512],
                        out_proj[:, qn * 512:(qn + 1) * 512])

            stage_pre(0)
            for img in range(n_imgs):
                last = img == n_imgs - 1
                if last:
                    # no next image: run attention alone, fire the normalize,
                    # then fill the exp-drain tail with the conv
                    for r in range(16):
                        stage_round(img, r)
                    if pend[0] is not None:
                        normalize(*pend[0])
                        pend[0] = None
                    for ci in range(len(CHUNKS)):
                        stage_conv(img, ci)
                else:
                    for r in range(15):
                        stage_round(img, r)
                        if r % 5 == 1:
                            stage_conv(img, r // 5)
                    if img + 2 < n_imgs:
                        stage_in(img + 2)
                    stage_round(img, 15)
                    stage_pre(img + 1)
                stage_proj(img)
                del imgs[img]

    return nc


_NC = None


def _get_nc():
    global _NC
    if _NC is None:
        _NC = build_nc()
        _NC.compile()
    return _NC


def kernel(**inputs):
    from concourse.bass_utils import run_bass_kernel_spmd

    nc = _get_nc()
    x = np.asarray(inputs["x"], np.float32).reshape(B, C, HW)
    conv_w = np.ascontiguousarray(np.asarray(inputs["conv_w"], np.float32).reshape(9, C, CONV_CO))
    conv_b = np.ascontiguousarray(np.asarray(inputs["conv_b"], np.float32).reshape(1, CONV_CO))
    qkv_w = np.ascontiguousarray(np.asarray(inputs["qkv_w"], np.float32))
    qkv_b = np.ascontiguousarray(np.asarray(inputs["qkv_b"], np.float32).reshape(1, 2 * DK + DV))
    proj_w = np.ascontiguousarray(np.asarray(inputs["proj_w"], np.float32))
    proj_b = np.ascontiguousarray(np.asarray(inputs["proj_b"], np.float32).reshape(1, DV))

    in_maps = []
    for i in range(N_CORES):
        in_maps.append({
            "x": np.ascontiguousarray(x[i * BL:(i + 1) * BL]),
            "conv_w": conv_w, "conv_b": conv_b,
            "qkv_w": qkv_w, "qkv_b": qkv_b,
            "proj_w": proj_w, "proj_b": proj_b,
        })
    res = run_bass_kernel_spmd(nc, in_maps, core_ids=list(range(N_CORES)))
    outs = [np.asarray(res.results[i]["out"]).reshape(BL, CO, H, W)
            for i in range(N_CORES)]
    return np.concatenate(outs, axis=0).astype(np.float32)


if __name__ == "__main__":
    nc = build_nc()
    nc.compile()
    print("built ok; instructions:", len(nc.inst_map))

